# revision 10
# baseline (speedup 1.0000x reference)
"""DecouplingGCN fused kernel for 8 Trainium2 NeuronCores.

Data-parallel over batch N (4 samples/core). Sync-BN handled by:
  L1 (device): per-core partial stats of x (S2 = x^T x, per-vertex sums).
  Host: reduces partials, folds BN1 analytically into the adjacency
      (A2 = a1 * L2-row-normalized A) + per-(c,w) const, and computes all
      first moments of u0 analytically (only sum(u0^2) needs the device).
  L2 (device): slab = (n, t128) -> (128, 19*256): x^T via PE transpose ->
      y = x@W (f16 matmul, fp32 accum) -> Y (t,(v,d)) -> per c-pair PE
      transpose to (c2,v,k)-rows -> z = YT^T @ A2block (z natural) ->
      u0 = x + z (f16, SBUF-resident) -> sum(u0^2) partials (ones-matmuls)
      -> 19KB AllReduce -> BN2 affine built on device -> relu sweep -> out.
"""
import functools
import numpy as np

import concourse.bacc as bacc
import concourse.mybir as mybir
from concourse import tile
from concourse.bass_utils import run_bass_kernel_spmd

F16 = mybir.dt.float16
F32 = mybir.dt.float32

NC_ = 8
NLOC = 4
N, T, V, D, SUB, GROUPS = 32, 256, 19, 256, 3, 8
SLABS = NLOC * (T // 128)   # 8 slabs of (128, V*D) per core
CW = V * D                  # 4864
M_TOT = N * T * V
NT_G = N * T
EPS = 1e-5
CORES = list(range(NC_))
AF = mybir.ActivationFunctionType
ALU = mybir.AluOpType


@functools.lru_cache(maxsize=1)
def _build_l1():
    nc = bacc.Bacc("TRN2", target_bir_lowering=False, debug=False, num_devices=NC_)
    x = nc.dram_tensor("x", [SLABS, 128, CW], F32, kind="ExternalInput")
    s2o = nc.dram_tensor("s2part", [128, 512], F32, kind="ExternalOutput")
    sxo = nc.dram_tensor("sxvpart", [128, 38], F32, kind="ExternalOutput")
    with tile.TileContext(nc) as tc:
        with (
            tc.tile_pool(name="cst", bufs=1) as cst,
            tc.tile_pool(name="xp", bufs=3) as xp,
            tc.tile_pool(name="ps", bufs=1, space="PSUM") as ps,
        ):
            ones1 = cst.tile([128, 1], F16)
            nc.vector.memset(ones1[:], 1.0)
            s2p0 = ps.tile([128, 256], F32)
            s2p1 = ps.tile([128, 256], F32)
            sxvp = ps.tile([128, 38], F32)
            for s in range(SLABS):
                xsb = xp.tile([128, CW], F16, tag="x")
                nc.gpsimd.dma_start(xsb[:], x[s, :, :])
                for v in range(V):
                    base = v * 256
                    rhs = xsb[:, base : base + 256]
                    for h in (0, 1):
                        lhs = xsb[:, base + h * 128 : base + (h + 1) * 128]
                        nc.tensor.matmul(
                            (s2p0 if h == 0 else s2p1)[:, :], lhs, rhs,
                            start=(s == 0 and v == 0),
                            stop=(s == SLABS - 1 and v == V - 1),
                        )
                        j = v * 2 + h
                        nc.tensor.matmul(
                            sxvp[:, j : j + 1], lhs, ones1[:, :],
                            start=(s == 0), stop=(s == SLABS - 1),
                        )
            s2sb = cst.tile([128, 512], F32)
            nc.vector.tensor_copy(s2sb[:, 0:256], s2p0[:, :])
            nc.vector.tensor_copy(s2sb[:, 256:512], s2p1[:, :])
            sxsb = cst.tile([128, 38], F32)
            nc.vector.tensor_copy(sxsb[:], sxvp[:])
            nc.sync.dma_start(s2o[:, :], s2sb[:])
            nc.sync.dma_start(sxo[:, :], sxsb[:])
    nc.compile()
    return nc


@functools.lru_cache(maxsize=2)
def _build_l2(debug=False):
    nc = bacc.Bacc("TRN2", target_bir_lowering=False, debug=False, num_devices=NC_)
    x = nc.dram_tensor("x", [SLABS, 128, CW], F32, kind="ExternalInput")
    w16 = nc.dram_tensor("w16", [2, 128, 768], F16, kind="ExternalInput")
    a2t = nc.dram_tensor("a2t", [114, CW], F16, kind="ExternalInput")
    ident = nc.dram_tensor("ident", [128, 128], F16, kind="ExternalInput")
    ctile = nc.dram_tensor("consttile", [128, 38], F32, kind="ExternalInput")
    corrt = nc.dram_tensor("corrtile", [128, 38], F32, kind="ExternalInput")
    m2t_d = nc.dram_tensor("m2t", [128, 2], F32, kind="ExternalInput")
    g2t_d = nc.dram_tensor("g2t", [128, 2], F32, kind="ExternalInput")
    b2t_d = nc.dram_tensor("b2t", [128, 2], F32, kind="ExternalInput")
    out = nc.dram_tensor("out", [SLABS, 128, CW], F32, kind="ExternalOutput")
    u0dump = (nc.dram_tensor("u0dump", [SLABS, 128, CW], F32, kind="ExternalOutput")
              if debug else None)
    sqdump = (nc.dram_tensor("sqdump", [128, 38], F32, kind="ExternalOutput")
              if debug else None)
    sqloc = (nc.dram_tensor("sqloc", [128, 38], F32, kind="ExternalOutput")
             if debug else None)
    scale_dr = nc.dram_tensor("scale_dr", [V, 2, 128], F32)
    shift_dr = nc.dram_tensor("shift_dr", [V, 2, 128], F32)

    with tile.TileContext(nc) as tc:
        with (
            tc.tile_pool(name="cst", bufs=1) as cst,
            tc.tile_pool(name="smal", bufs=1) as smal,
            tc.tile_pool(name="xp", bufs=2) as xp,
            tc.tile_pool(name="u0p", bufs=1) as u0p,
            tc.tile_pool(name="yp", bufs=1) as yp,
            tc.tile_pool(name="xtp", bufs=3) as xtp,
            tc.tile_pool(name="ytp", bufs=3) as ytp,
            tc.tile_pool(name="t1p", bufs=1) as t1p,
            tc.tile_pool(name="swp", bufs=1) as swp,
            tc.tile_pool(name="scc", bufs=2) as scc,
            tc.tile_pool(name="yps", bufs=1, space="PSUM") as yps,
            tc.tile_pool(name="tps", bufs=2, space="PSUM") as tps,
            tc.tile_pool(name="zps", bufs=2, space="PSUM") as zps,
            tc.tile_pool(name="qps", bufs=1, space="PSUM") as qps,
            tc.tile_pool(name="dram", bufs=1, space="DRAM") as dram,
        ):
            # ---- constants ----
            wsb0 = cst.tile([128, 768], F16)
            wsb1 = cst.tile([128, 768], F16)
            nc.sync.dma_start(wsb0[:], w16[0, :, :])
            nc.sync.dma_start(wsb1[:], w16[1, :, :])
            a2sb = cst.tile([114, CW], F16)
            nc.sync.dma_start(a2sb[:], a2t[:, :])
            idsb = cst.tile([128, 128], F16)
            nc.sync.dma_start(idsb[:], ident[:, :])
            ones1 = cst.tile([128, 1], F16)
            nc.vector.memset(ones1[:], 1.0)
            ones19 = cst.tile([128, 19], F32)
            nc.vector.memset(ones19[:], 1.0)
            onescol = cst.tile([1, 128], F32)
            nc.vector.memset(onescol[:], 1.0)
            ctsb = smal.tile([128, 38], F32)
            nc.sync.dma_start(ctsb[:], ctile[:, :])
            corrsb = smal.tile([128, 38], F32)
            nc.sync.dma_start(corrsb[:], corrt[:, :])
            m2sb = smal.tile([128, 2], F32)
            nc.sync.dma_start(m2sb[:], m2t_d[:, :])
            g2sb = smal.tile([128, 2], F32)
            nc.sync.dma_start(g2sb[:], g2t_d[:, :])
            b2sb = smal.tile([128, 2], F32)
            nc.sync.dma_start(b2sb[:], b2t_d[:, :])

            u0 = u0p.tile([128, SLABS * CW], F16)
            sq_sb = smal.tile([128, 38], F32)
            nc.vector.memset(sq_sb[:], 0.0)

            for s in range(SLABS):
                xsb = xp.tile([128, CW], F16, tag="x")
                nc.gpsimd.dma_start(xsb[:], x[s, :, :])
                ysb = yp.tile([128, V * 768], F16, tag="y")
                # ---- y = x @ W per vertex ----
                for v in range(V):
                    base = v * 256
                    xtps = tps.tile([128, 512], F16, tag="tp")
                    for h in (0, 1):
                        nc.tensor.transpose(
                            xtps[0:128, h * 128 : (h + 1) * 128],
                            xsb[:, base + h * 128 : base + (h + 1) * 128],
                            idsb[:],
                        )
                    xt = xtp.tile([128, 256], F16, tag="xt")
                    nc.vector.tensor_copy(xt[:], xtps[:, 0:256])
                    yh0 = yps.tile([128, 512], F32, tag="h0")
                    yh1 = yps.tile([128, 256], F32, tag="h1")
                    for h in (0, 1):
                        wh = wsb0 if h == 0 else wsb1
                        lhs = xt[:, h * 128 : (h + 1) * 128]
                        nc.tensor.matmul(yh0[:, :], lhs, wh[:, 0:512],
                                         start=(h == 0), stop=(h == 1))
                        nc.tensor.matmul(yh1[:, :], lhs, wh[:, 512:768],
                                         start=(h == 0), stop=(h == 1))
                    # scatter d=(k,c) -> free pos c*57 + v*3 + k
                    y0ap = ysb[:].rearrange(
                        "p (c vk) -> p c vk", vk=57)[:, :, v * 3 : v * 3 + 2]
                    y0ap = y0ap.rearrange("p c k -> p k c")
                    nc.vector.tensor_copy(
                        y0ap, yh0[:].rearrange("p (k c) -> p k c", k=2))
                    y1ap = ysb[:].rearrange(
                        "p (c vk) -> p c vk", vk=57)[:, :, v * 3 + 2]
                    nc.scalar.activation(y1ap, yh1[:, :], AF.Copy)
                xv = xsb[:].rearrange("p (w c) -> p c w", w=V)
                uv = u0[:, s * CW : (s + 1) * CW].rearrange(
                    "p (w c) -> p c w", w=V)
                # ---- z = A2-einsum; u0 = x + z ----
                for g in range(16):
                    zt = zps.tile([128, 512], F32, tag="z")
                    for j4 in range(2):
                        tp = tps.tile([128, 512], F16, tag="tp")
                        for jj in range(4):
                            cp = g * 8 + j4 * 4 + jj
                            nc.tensor.transpose(
                                tp[0:114, jj * 128 : (jj + 1) * 128],
                                ysb[:, cp * 114 : (cp + 1) * 114],
                                idsb[:],
                            )
                        yt = ytp.tile([128, 512], F16, tag="yt")
                        nc.vector.tensor_copy(yt[0:114, :], tp[0:114, :])
                        for jj in range(4):
                            cp = g * 8 + j4 * 4 + jj
                            q = (j4 * 4 + jj) * 64
                            nc.tensor.matmul(
                                zt[:, q : q + 38],
                                yt[0:114, jj * 128 : (jj + 1) * 128],
                                a2sb[:, cp * 38 : (cp + 1) * 38],
                                start=True, stop=True,
                            )
                    uap = uv[:, 16 * g : 16 * g + 16, :].rearrange(
                        "p (j c2) w -> p j c2 w", j=8)
                    xap = xv[:, 16 * g : 16 * g + 16, :].rearrange(
                        "p (j c2) w -> p j c2 w", j=8)
                    zap = zt[:].rearrange("p (j q) -> p j q", j=8)[:, :, 0:38]
                    zap = zap.rearrange("p j (c2 w) -> p j c2 w", c2=2)
                    nc.vector.tensor_add(uap, xap, zap)
                # ---- sum(u0^2) partials ----
                sqs = t1p.tile([128, CW], F16, tag="t1")
                for hh in (0, 1):
                    nc.scalar.square(
                        sqs[:, hh * 2432 : (hh + 1) * 2432],
                        u0[:, s * CW + hh * 2432 : s * CW + (hh + 1) * 2432],
                    )
                if debug:
                    nc.gpsimd.dma_start(
                        u0dump[s, :, :], u0[:, s * CW : (s + 1) * CW])
                sqp_s = qps.tile([128, 38], F32, tag="sq")
                for j in range(38):
                    nc.tensor.matmul(
                        sqp_s[:, j : j + 1],
                        sqs[:, j * 128 : (j + 1) * 128],
                        ones1[:, :],
                        start=True, stop=True,
                    )
                nc.vector.tensor_add(sq_sb[:], sq_sb[:], sqp_s[:])

            # ---- AllReduce sum(u0^2) ----
            arin = dram.tile([128, 38], F32)
            arout = dram.tile([128, 38], F32)
            arsb = sq_sb
            if debug:
                nc.sync.dma_start(sqloc[:, :], arsb[:])
            nc.sync.dma_start(arin[:], arsb[:])
            nc.gpsimd.collective_compute(
                "AllReduce", ALU.add,
                replica_groups=[CORES],
                ins=[arin.opt()], outs=[arout.opt()],
            )
            sq_g = smal.tile([128, 38], F32)
            nc.sync.dma_start(sq_g[:], arout[:])
            if debug:
                nc.sync.dma_start(sqdump[:, :], sq_g[:])

            # ---- BN2 affine (per (c,w)) ----
            sq1 = smal.tile([128, 38], F32)
            nc.vector.tensor_add(sq1[:], sq_g[:], corrsb[:])
            e2 = smal.tile([128, 2], F32)
            for h in (0, 1):
                nc.vector.tensor_reduce(
                    e2[:, h : h + 1], sq1[:, h::2],
                    axis=mybir.AxisListType.X, op=ALU.add,
                )
            nc.vector.tensor_scalar_mul(e2[:], e2[:], 1.0 / M_TOT)
            m2sq = smal.tile([128, 2], F32)
            nc.scalar.square(m2sq[:], m2sb[:])
            var2 = smal.tile([128, 2], F32)
            nc.vector.tensor_sub(var2[:], e2[:], m2sq[:])
            nc.vector.tensor_scalar_add(var2[:], var2[:], EPS)
            inv = smal.tile([128, 2], F32)
            nc.vector.reciprocal(inv[:], var2[:])
            scalev = smal.tile([128, 2], F32)
            nc.scalar.sqrt(scalev[:], inv[:])
            nc.vector.tensor_mul(scalev[:], scalev[:], g2sb[:])
            scale38 = smal.tile([128, 38], F32)
            shift38 = smal.tile([128, 38], F32)
            for h in (0, 1):
                nc.vector.tensor_scalar_mul(
                    scale38[:, h::2], ones19[:], scalev[:, h : h + 1])
                nc.vector.tensor_scalar(
                    shift38[:, h::2], ctsb[:, h::2],
                    m2sb[:, h : h + 1], scalev[:, h : h + 1],
                    ALU.subtract, ALU.mult,
                )
                nc.vector.tensor_scalar_add(
                    shift38[:, h::2], shift38[:, h::2], b2sb[:, h : h + 1])
            # bounce to DRAM in (w,h,c) order; broadcast via K=1 matmul
            for h in (0, 1):
                nc.sync.dma_start(
                    scale_dr[:, h, :].rearrange("w c -> c w").opt(),
                    scale38[:, h::2].opt(),
                )
                nc.sync.dma_start(
                    shift_dr[:, h, :].rearrange("w c -> c w").opt(),
                    shift38[:, h::2].opt(),
                )
            sc_slab = swp.tile([128, CW], F16, tag="scs")
            sh_slab = swp.tile([128, CW], F16, tag="shs")
            scale_fl = scale_dr[:, :, :].rearrange("w h c -> (w h c)")
            shift_fl = shift_dr[:, :, :].rearrange("w h c -> (w h c)")
            for c0 in range(0, CW, 512):
                cn = min(512, CW - c0)
                for fl, slab in ((scale_fl, sc_slab), (shift_fl, sh_slab)):
                    ch = scc.tile([1, 512], F32, tag="ch")
                    nc.sync.dma_start(ch[0:1, 0:cn], fl[c0 : c0 + cn])
                    bp = zps.tile([128, 512], F32, tag="z")
                    nc.tensor.matmul(bp[:, 0:cn], onescol[:, :],
                                     ch[0:1, 0:cn], start=True, stop=True)
                    nc.vector.tensor_copy(slab[:, c0 : c0 + cn], bp[:, 0:cn])

            # ---- sweep: out = relu(u0*scale + shift) ----
            for s in range(SLABS):
                us = u0[:, s * CW : (s + 1) * CW]
                t1 = t1p.tile([128, CW], F16, tag="t1")
                nc.vector.tensor_mul(t1[:], us, sc_slab[:])
                nc.vector.tensor_add(us, t1[:], sh_slab[:])
                nc.vector.tensor_scalar_max(us, us, 0.0)
                nc.gpsimd.dma_start(out[s, :, :], us)
    nc.compile()
    return nc


def _host_fold(S2, Sxv, para, W, b, g1, be1):
    S2 = S2.astype(np.float64)
    Sxv = Sxv.astype(np.float64)
    W = W.astype(np.float64)
    b = b.astype(np.float64)
    s1 = Sxv.sum(axis=0)
    Ws1 = W.T @ s1
    m1 = (Ws1 + M_TOT * b) / M_TOT
    q = np.einsum("cd,ce,ed->d", W, S2, W)
    E2 = (q + 2.0 * b * Ws1 + M_TOT * b * b) / M_TOT
    var1 = E2 - m1 * m1
    a1 = g1.astype(np.float64) / np.sqrt(var1 + EPS)
    c1 = be1.astype(np.float64) + (b - m1) * a1
    A = np.tile(para.astype(np.float64), (1, D // GROUPS, 1, 1))
    A = A / (np.linalg.norm(A, axis=-1, keepdims=True) + 1e-4)
    A2 = A * a1.reshape(SUB, D)[:, :, None, None]
    const = np.einsum("kc,kcvw->cw", c1.reshape(SUB, D), A)
    Sy = (Sxv @ W).reshape(V, SUB, D)
    sum_z = np.einsum("kcvw,vkc->cw", A2, Sy)
    sum_u0 = sum_z + Sxv.T
    return A2, const, sum_u0


def _cw_tile(vec_cw):
    t = np.zeros((128, 38), np.float32)
    for h in (0, 1):
        t[:, h::2] = vec_cw[h * 128 : (h + 1) * 128, :].astype(np.float32)
    return t


def _c_tile(vec_c):
    return np.ascontiguousarray(
        np.stack([vec_c[0:128], vec_c[128:256]], axis=1)).astype(np.float32)


def kernel(x, para, linear_weight, linear_bias, bn_gamma, bn_beta,
           bn1_gamma, bn1_beta):
    x = np.asarray(x, np.float32)
    para = np.asarray(para, np.float32)
    W = np.asarray(linear_weight, np.float32)
    b = np.asarray(linear_bias, np.float32).reshape(-1)
    g1 = np.asarray(bn_gamma, np.float32)
    be1 = np.asarray(bn_beta, np.float32)
    g2 = np.asarray(bn1_gamma, np.float32)
    be2 = np.asarray(bn1_beta, np.float32)

    shards = [
        np.ascontiguousarray(x[c * NLOC : (c + 1) * NLOC]).reshape(SLABS, 128, CW)
        for c in range(NC_)
    ]

    nc1 = _build_l1()
    r1 = run_bass_kernel_spmd(nc1, [{"x": s} for s in shards], CORES)
    S2 = np.zeros((256, 256), np.float64)
    Sxv = np.zeros((V, 256), np.float64)
    for c in range(NC_):
        s2p = r1.results[c]["s2part"].astype(np.float64)
        S2[0:128] += s2p[:, 0:256]
        S2[128:256] += s2p[:, 256:512]
        sx = r1.results[c]["sxvpart"].astype(np.float64)
        for h in (0, 1):
            Sxv[:, h * 128 : (h + 1) * 128] += sx[:, h::2].T

    A2, const, sum_u0 = _host_fold(S2, Sxv, para, W, b, g1, be1)

    a2t = np.zeros((114, CW), np.float16)
    for c2 in range(2):
        rows = (c2 * 57
                + (np.arange(V)[:, None] * 3 + np.arange(SUB)[None, :]).reshape(-1))
        for cp in range(128):
            c = 2 * cp + c2
            blk = A2[:, c, :, :].transpose(1, 0, 2).reshape(57, V)  # (v,k),w
            a2t[rows, cp * 38 + c2 * 19 : cp * 38 + (c2 + 1) * 19] = (
                blk.astype(np.float16))

    m2 = (sum_u0.sum(axis=1) + NT_G * const.sum(axis=1)) / M_TOT
    corr = 2.0 * const * sum_u0 + NT_G * const * const

    ins2 = {
        "w16": np.stack([W[0:128], W[128:256]]).astype(np.float16),
        "a2t": a2t,
        "ident": np.eye(128, dtype=np.float16),
        "consttile": _cw_tile(const),
        "corrtile": _cw_tile(corr),
        "m2t": _c_tile(m2),
        "g2t": _c_tile(g2.astype(np.float64)),
        "b2t": _c_tile(be2.astype(np.float64)),
    }
    nc2 = _build_l2()
    in_maps = [dict(ins2, x=shards[c]) for c in range(NC_)]
    r2 = run_bass_kernel_spmd(nc2, in_maps, CORES)

    out = np.empty((N, T, V, D), np.float32)
    for c in range(NC_):
        o = r2.results[c]["out"].reshape(NLOC, T, V, D)
        out[c * NLOC : (c + 1) * NLOC] = o
    return out
